# revision 58
# baseline (speedup 1.0000x reference)
"""Multi-head causal attention (B=2, T=2048, C=1024, H=16) on 8 trn2 NeuronCores.

Sharding: 2 heads per core (tensor-parallel over heads), both batch elements
on every core. Per core:
  1. qkv projection for its 2 heads, j-chunked (512 tokens at a time) so the
     first matmul fires as soon as the first strided x chunk lands; q^T,k^T
     kept in [d, t] layout, v transposed to [t, d] (fp16 matmuls, fp32 PSUM).
  2. Flash-style causal attention in the S^T = k q^T layout: exp on ScalarE
     straight out of PSUM, row-sums folded into the P@V matmul via a ones
     column appended to V, division by the row-sum on VectorE.
  3. Pipelined exchange: one AllToAll per batch (collective instances pay
     large one-time channel-init costs charged to the first collective, so
     the count is kept minimal). Batch 0's exchange and its output
     projection hide under batch 1's attention; only batch 1's exchange +
     projection are exposed. Core r receives rows [256r, 256r+256) of each
     batch.
Host side shards/transposes/casts inputs (fp16) and reassembles the output.
"""

import sys

import numpy as np

if "/opt/trn_rl_repo" not in sys.path:
    sys.path.insert(0, "/opt/trn_rl_repo")

B, T, C, H, D = 2, 2048, 1024, 16, 64
NCORES = 8
HPC = H // NCORES          # heads per core = 2
CW = HPC * D               # per-core channel width = 128
KT = C // 128              # k tiles = 8
TT = T // 128              # t tiles = 16
SHARD = (B * T) // NCORES  # output rows per core = 512
SCALE = 1.0 / float(np.sqrt(C))

_CACHE = {}
LAST_EXEC_NS = None


def _build_nc():
    import concourse.mybir as mybir
    import concourse.tile as tile
    from concourse import bacc
    from concourse.masks import make_identity, make_upper_triangular

    f32 = mybir.dt.float32
    f16 = mybir.dt.float16

    nc = bacc.Bacc("TRN2", target_bir_lowering=False, debug=False,
                   num_devices=NCORES)

    xT = nc.dram_tensor("xT", [B, 128, KT * T], f16, kind="ExternalInput")
    wq = nc.dram_tensor("wq", [128, KT * CW], f16, kind="ExternalInput")
    wk = nc.dram_tensor("wk", [128, KT * CW], f16, kind="ExternalInput")
    wv = nc.dram_tensor("wv", [128, KT * CW], f16, kind="ExternalInput")
    wp = nc.dram_tensor("wp", [128, KT * C], f16, kind="ExternalInput")
    bq = nc.dram_tensor("bq", [CW, 1], f32, kind="ExternalInput")
    bk = nc.dram_tensor("bk", [CW, 1], f32, kind="ExternalInput")
    bv = nc.dram_tensor("bv", [CW, 1], f32, kind="ExternalInput")
    bp = nc.dram_tensor("bp", [1, C], f32, kind="ExternalInput")
    y = nc.dram_tensor("y", [SHARD, C], f32, kind="ExternalOutput")

    with tile.TileContext(nc) as tc:
        with (
            tc.tile_pool(name="const", bufs=1) as const,
            tc.tile_pool(name="dram", bufs=1, space="DRAM") as dram,
            tc.tile_pool(name="xtp", bufs=2) as xtp,
            tc.tile_pool(name="wqkv", bufs=1) as wqkvp,
            tc.tile_pool(name="qkv", bufs=1) as qkvp,
            tc.tile_pool(name="pt", bufs=5) as ptp,
            tc.tile_pool(name="otp", bufs=1) as otp,
            tc.tile_pool(name="sm", bufs=1) as smp,
            tc.tile_pool(name="proj", bufs=1) as projp,
            tc.tile_pool(name="ysb", bufs=2) as ysbp,
            tc.tile_pool(name="ps_gen", bufs=2, space="PSUM") as psg,
            tc.tile_pool(name="ps_s", bufs=2, space="PSUM") as ps_s,
            tc.tile_pool(name="ps_o", bufs=1, space="PSUM") as ps_o,
        ):
            # ---- collective warm-up (channel init overlaps compute) ----
            warm_i = dram.tile([8, 16], f32, name="warm_i")
            warm_o = dram.tile([8, 16], f32, name="warm_o")
            wtile = const.tile([8, 16], f32, name="wtile")
            nc.vector.memset(wtile[:], 0.0)
            nc.sync.dma_start(warm_i[:], wtile[:])
            nc.gpsimd.collective_compute(
                "AllToAll", mybir.AluOpType.bypass,
                replica_groups=[list(range(NCORES))],
                ins=[warm_i[:].opt()], outs=[warm_o[:].opt()],
            )

            # ---- weights first (small), then x chunks, then wp ----
            wq_sb = wqkvp.tile([128, KT * CW], f16, name="wq_sb")
            wk_sb = wqkvp.tile([128, KT * CW], f16, name="wk_sb")
            wv_sb = wqkvp.tile([128, KT * CW], f16, name="wv_sb")

            # x chunks: xt[b][j] holds tokens [512j, 512j+512) for all KT
            # c-chunks, a-major: [128, KT*512]
            xt_tiles = {0: [], 1: []}
            xT_r = [xT[b].rearrange("p (a t) -> p a t", a=KT) for b in range(B)]

            def load_x(b, j):
                xt = xtp.tile([128, KT * 512], f16, name=f"xt{b}_{j}",
                              tag=f"xt{j}")
                nc.sync.dma_start(
                    xt[:].rearrange("p (a w) -> p a w", a=KT),
                    xT_r[b][:, :, 512 * j : 512 * (j + 1)],
                )
                xt_tiles[b].append(xt)

            nc.sync.dma_start(wq_sb[:], wq[:])
            load_x(0, 0)
            load_x(0, 1)
            nc.sync.dma_start(wk_sb[:], wk[:])
            nc.sync.dma_start(wv_sb[:], wv[:])
            load_x(0, 2)
            load_x(0, 3)

            bq_t = const.tile([CW, 1], f32, name="bq_t")
            bk_t = const.tile([CW, 1], f32, name="bk_t")
            bv_t = const.tile([CW, 1], f32, name="bv_t")
            nc.sync.dma_start(bq_t[:], bq[:])
            nc.sync.dma_start(bk_t[:], bk[:])
            nc.sync.dma_start(bv_t[:], bv[:])
            bp_row = const.tile([1, C], f32, name="bp_row")
            nc.sync.dma_start(bp_row[:], bp[:])

            # batch-1 x and wp early so the HBM queues are quiet during the
            # batch-0 exchange
            for j in range(4):
                load_x(1, j)
            wp_sb = projp.tile([128, KT * C], f16, name="wp_sb")
            nc.sync.dma_start(wp_sb[:], wp[:])

            # ---- constants ----
            trimask = const.tile([128, 128], f16, name="trimask")
            make_upper_triangular(nc, trimask[:], val=1.0, diag=True)
            ident = const.tile([128, 128], f16, name="ident")
            make_identity(nc, ident[:])
            bpb = const.tile([128, C], f32, name="bpb")
            nc.gpsimd.partition_broadcast(bpb[:], bp_row[:])

            qT_sb, kT_sb, v_sb, ot_sb, r_all = {}, {}, {}, {}, {}
            # asymmetric exchange split: "A" carries all of batch 0 (slot r
            # cols 0:256 = b0 rows [256r, 256r+256)) plus batch 1's first
            # half (cols 256:384 = b1 rows [128r, 128r+128)); "B" carries
            # batch 1's second half (b1 rows [1024+128r, 1024+128r+128)).
            a2a_in = {
                "A": dram.tile([NCORES, 128, 384], f16, name="a2a_in_A"),
                "B": dram.tile([NCORES, 128, 128], f16, name="a2a_in_B"),
            }
            a2a_out = {
                "A": dram.tile([NCORES, 128, 384], f16, name="a2a_out_A"),
                "B": dram.tile([NCORES, 128, 128], f16, name="a2a_out_B"),
            }

            vT_sb = {}

            def emit_qkv_chunk(b, j):
                """qkv projection + v transposes for token chunk j of batch
                b. Emitted chunk-wise so attention chunk j can start as soon
                as its (causal) q/k/v prefix exists — attention work fills
                the x-DMA ramp instead of the PE idling."""
                if j == 0:
                    qT_sb[b] = qkvp.tile([128, T], f16, name=f"qT{b}")
                    kT_sb[b] = qkvp.tile([128, T], f16, name=f"kT{b}")
                    vT_sb[b] = qkvp.tile([128, T], f16, name=f"vT{b}")
                    v_sb[b] = []
                qT_b, kT_b, vT_b = qT_sb[b], kT_sb[b], vT_sb[b]
                xt = xt_tiles[b][j]
                for dst, w_sb, bias in (
                    (qT_b, wq_sb, bq_t), (kT_b, wk_sb, bk_t),
                    (vT_b, wv_sb, bv_t),
                ):
                    ps = psg.tile([128, 512], f32, name="ps_qk",
                                  tag="ps_gen")
                    for a in range(KT):
                        nc.tensor.matmul(
                            ps[:],
                            w_sb[:, CW * a : CW * (a + 1)],
                            xt[:, 512 * a : 512 * (a + 1)],
                            start=(a == 0), stop=(a == KT - 1),
                        )
                    nc.vector.tensor_scalar_add(
                        dst[:, 512 * j : 512 * (j + 1)], ps[:], bias[:]
                    )
                for m in range(4 * j, 4 * j + 4):
                    vt = qkvp.tile([128, 2 * (D + 1)], f16,
                                   name=f"v{b}_{m}")
                    tps = psg.tile([128, 128], f16, name="ps_tr",
                                   tag="ps_gen")
                    nc.tensor.transpose(
                        tps[:], vT_b[:, 128 * m : 128 * (m + 1)], ident[:]
                    )
                    nc.vector.tensor_copy(
                        vt[:].rearrange("p (a m) -> p a m", a=2)[:, :, 0:D],
                        tps[:].rearrange("p (a m) -> p a m", a=2),
                    )
                    nc.vector.memset(vt[:, D : D + 1], 1.0)
                    nc.vector.memset(vt[:, 2 * D + 1 : 2 * D + 2], 1.0)
                    v_sb[b].append(vt)

            def emit_attn_chunk(b, j, last=False):
                """Attention for q-chunk j of batch b; stages its rows into
                the exchange buffers."""
                if j == 0:
                    ot = otp.tile([128, T], f16, name=f"ot{b}")
                    ot_sb[b] = ot
                    ra = smp.tile([1, 4096], f32, name=f"r_all{b}",
                                  tag=f"r_all{b}")
                    r_all[b] = ra
                ot, ra = ot_sb[b], r_all[b]
                o_ps = [
                    ps_o.tile([65, 512], f32, name=f"o{h}", tag=f"o{h}")
                    for h in range(2)
                ]
                ilast = 4 * (j + 1) - 1
                for i in range(4 * (j + 1)):
                    off = max(0, 128 * i - 512 * j)
                    s_ps = ps_s.tile([128, 1024], f32, name="s_ps", tag="s")
                    pt = ptp.tile([128, 1024], f16, name="pt", tag="pt")
                    for h in range(2):
                        nc.tensor.matmul(
                            s_ps[:, 512 * h + off : 512 * (h + 1)],
                            kT_sb[b][64 * h : 64 * h + 64,
                                     128 * i : 128 * (i + 1)],
                            qT_sb[b][64 * h : 64 * h + 64,
                                     512 * j + off : 512 * (j + 1)],
                            start=True, stop=True,
                        )
                    nc.scalar.activation(
                        pt[:].rearrange("p (g w) -> p g w", g=2)[:, :, off:512],
                        s_ps[:].rearrange("p (g w) -> p g w", g=2)[:, :, off:512],
                        mybir.ActivationFunctionType.Exp,
                        scale=SCALE,
                    )
                    if 4 * j <= i:
                        for h in range(2):
                            nc.vector.tensor_tensor(
                                pt[:, 512 * h + off : 512 * h + off + 128],
                                pt[:, 512 * h + off : 512 * h + off + 128],
                                trimask[:],
                                op=mybir.AluOpType.mult,
                            )
                    for h in range(2):
                        nc.tensor.matmul(
                            o_ps[h][0:65, off:512],
                            v_sb[b][i][:, (D + 1) * h : (D + 1) * (h + 1)],
                            pt[:, 512 * h + off : 512 * (h + 1)],
                            start=(i == 0), stop=(i == ilast),
                        )
                # rowsums + division + staging for chunk j
                for h in range(2):
                    idx = 2 * j + h
                    nc.vector.tensor_copy(
                        ra[0:1, 512 * idx : 512 * (idx + 1)],
                        o_ps[h][64:65, :],
                    )
                    if not last:
                        nc.vector.tensor_copy(
                            ot[64 * h : 64 * h + 64, 512 * j : 512 * (j + 1)],
                            o_ps[h][0:64, :],
                        )
                rs = ra[0:1, 1024 * j : 1024 * j + 1024]
                nc.vector.reciprocal_approx_fast(rs, rs)
                rb = smp.tile([128, 1024], f32, name="rb", tag="rb", bufs=2)
                nc.gpsimd.partition_broadcast(rb[:], rs)
                for h in range(2):
                    sl = ot[64 * h : 64 * h + 64, 512 * j : 512 * (j + 1)]
                    nc.vector.tensor_tensor(
                        sl,
                        # last chunk: evict+divide fused straight from PSUM
                        # (no successor waits on the o_ps banks)
                        o_ps[h][0:64, :] if last else sl,
                        rb[64 * h : 64 * h + 64, 512 * h : 512 * (h + 1)],
                        op=mybir.AluOpType.mult,
                    )
                src = ot[:, 512 * j : 512 * (j + 1)]
                if b == 0:
                    nc.sync.dma_start(
                        a2a_in["A"][2 * j : 2 * j + 2, :, 0:256]
                            .rearrange("r p w -> p r w"),
                        src.rearrange("p (r w) -> p r w", r=2),
                    )
                elif j < 2:
                    nc.sync.dma_start(
                        a2a_in["A"][4 * j : 4 * j + 4, :, 256:384]
                            .rearrange("r p w -> p r w"),
                        src.rearrange("p (r w) -> p r w", r=4),
                    )
                else:
                    nc.sync.dma_start(
                        a2a_in["B"][4 * (j - 2) : 4 * (j - 2) + 4, :, :]
                            .rearrange("r p w -> p r w"),
                        src.rearrange("p (r w) -> p r w", r=4),
                    )

            def emit_cc(key):
                nc.gpsimd.collective_compute(
                    "AllToAll", mybir.AluOpType.bypass,
                    replica_groups=[list(range(NCORES))],
                    ins=[a2a_in[key][:].opt()], outs=[a2a_out[key][:].opt()],
                )

            yts_sb = {}

            def emit_proj_load(key, m):
                """One fused DMA for all 8 lhsT blocks of a proj tile (a
                single Sync-engine trigger, so it can't convoy the queue)."""
                yts = projp.tile([128, KT * 128], f16, name=f"yts{key}_{m}",
                                 tag="yts", bufs=4)
                # scalar-engine DMA queue: keeps this cc-gated load from
                # head-of-line blocking the sync queue's staging DMAs
                nc.scalar.dma_start(
                    yts[:].rearrange("p (k w) -> p k w", k=KT),
                    a2a_out[key][:, :, 128 * m : 128 * (m + 1)]
                        .rearrange("k p w -> p k w"),
                )
                yts_sb[(key, m)] = yts

            def emit_proj(key, m, r0):
                """Output projection for my y rows [r0, r0+128) from column
                block m of exchange `key`."""
                yts = yts_sb[(key, m)]
                ysb = ysbp.tile([128, C], f32, name="ysb", tag="ysb")
                for n in range(2):
                    ps = psg.tile([128, 512], f32, name="ps_y", tag="ps_gen")
                    for k in range(KT):
                        nc.tensor.matmul(
                            ps[:],
                            yts[:, 128 * k : 128 * (k + 1)],
                            wp_sb[:, C * k + 512 * n : C * k + 512 * (n + 1)],
                            start=(k == 0), stop=(k == KT - 1),
                        )
                    nc.vector.tensor_tensor(
                        ysb[:, 512 * n : 512 * (n + 1)],
                        ps[:],
                        bpb[:, 512 * n : 512 * (n + 1)],
                        op=mybir.AluOpType.add,
                    )
                    # per-half y write: the first half ships while the
                    # second half's matmuls run
                    nc.scalar.dma_start(
                        y[r0 : r0 + 128, 512 * n : 512 * (n + 1)],
                        ysb[:, 512 * n : 512 * (n + 1)],
                    )

            # schedule: batch 0's exchange hides under batch 1's attention;
            # batch 0's projection overlaps batch 1's exchange on the PE, so
            # only batch 1's projection is fully exposed.
            for j in range(4):
                emit_qkv_chunk(0, j)
                emit_attn_chunk(0, j)
            emit_qkv_chunk(1, 0)
            emit_attn_chunk(1, 0)
            emit_qkv_chunk(1, 1)
            emit_attn_chunk(1, 1)
            emit_cc("A")
            emit_qkv_chunk(1, 2)
            emit_attn_chunk(1, 2)
            emit_qkv_chunk(1, 3)
            emit_attn_chunk(1, 3, last=True)
            emit_cc("B")
            # sim-time floor: the compile-time scheduler underestimates the
            # collectives, so without this it hoists proj work above the last
            # chunk's post-chain/staging on shared engine queues, delaying
            # the final exchange's doorbell by ~30us at runtime.
            with tc.tile_wait_until(1):
                emit_proj_load("A", 0)
                emit_proj_load("A", 1)
                emit_proj_load("A", 2)
                emit_proj_load("B", 0)
                emit_proj("A", 0, 0)
                emit_proj("A", 1, 128)
                emit_proj("A", 2, 256)
                emit_proj("B", 0, 384)

    nc.compile()
    return nc


def _get_nc():
    if "nc" not in _CACHE:
        _CACHE["nc"] = _build_nc()
    return _CACHE["nc"]


def kernel(x, W_attn, b_attn, W_proj, b_proj, _trace=False):
    global LAST_EXEC_NS
    from concourse.bass_utils import run_bass_kernel_spmd

    x = np.asarray(x, np.float32)
    W_attn = np.asarray(W_attn, np.float32)
    b_attn = np.asarray(b_attn, np.float32)
    W_proj = np.asarray(W_proj, np.float32)
    b_proj = np.asarray(b_proj, np.float32)

    def pmajor(w):  # [C, M] -> [128, KT*M], k-tile a at cols [a*M:(a+1)*M]
        m = w.shape[1]
        return np.ascontiguousarray(
            w.reshape(KT, 128, m).transpose(1, 0, 2).reshape(128, KT * m)
        ).astype(np.float16)

    xT = np.transpose(x, (0, 2, 1))  # [B, C, T]
    xT16 = np.ascontiguousarray(
        xT.reshape(B, KT, 128, T).transpose(0, 2, 1, 3).reshape(B, 128, KT * T)
    ).astype(np.float16)
    wp16 = pmajor(W_proj)
    bp_h = np.ascontiguousarray(b_proj).reshape(1, C)

    in_maps = []
    for c in range(NCORES):
        s = slice(CW * c, CW * (c + 1))
        in_maps.append({
            "xT": xT16,
            "wq": pmajor(W_attn[:, s]),
            "wk": pmajor(W_attn[:, C:][:, s]),
            "wv": pmajor(W_attn[:, 2 * C:][:, s]),
            "wp": wp16,
            "bq": np.ascontiguousarray(b_attn[s]).reshape(CW, 1),
            "bk": np.ascontiguousarray(b_attn[C:][s]).reshape(CW, 1),
            "bv": np.ascontiguousarray(b_attn[2 * C:][s]).reshape(1, CW),
            "bp": bp_h,
        })

    nc = _get_nc()
    res = run_bass_kernel_spmd(nc, in_maps, list(range(NCORES)), trace=_trace)
    LAST_EXEC_NS = res.exec_time_ns

    # core c's y rows: [0,256) = batch 0 tokens [256c, 256c+256);
    # [256,384) = batch 1 tokens [128c, 128c+128);
    # [384,512) = batch 1 tokens [1024+128c, 1024+128c+128).
    out = np.empty((B, T, C), np.float32)
    for c in range(NCORES):
        yc = res.results[c]["y"]
        out[0, 256 * c : 256 * (c + 1), :] = yc[0:256, :]
        out[1, 128 * c : 128 * (c + 1), :] = yc[256:384, :]
        out[1, 1024 + 128 * c : 1024 + 128 * (c + 1), :] = yc[384:512, :]
    return out


# revision 60
# speedup vs baseline: 1.0919x; 1.0919x over previous
"""Multi-head causal attention (B=2, T=2048, C=1024, H=16) on 8 trn2 NeuronCores.

Sharding: 2 heads per core (tensor-parallel over heads), both batch elements
on every core. Per core:
  1. qkv projection for its 2 heads, j-chunked (512 tokens at a time) so the
     first matmul fires as soon as the first strided x chunk lands; q^T,k^T
     kept in [d, t] layout, v transposed to [t, d] (fp16 matmuls, fp32 PSUM).
  2. Flash-style causal attention in the S^T = k q^T layout: exp on ScalarE
     straight out of PSUM, row-sums folded into the P@V matmul via a ones
     column appended to V, division by the row-sum on VectorE.
  3. Pipelined exchange: one AllToAll per batch (collective instances pay
     large one-time channel-init costs charged to the first collective, so
     the count is kept minimal). Batch 0's exchange and its output
     projection hide under batch 1's attention; only batch 1's exchange +
     projection are exposed. Core r receives rows [256r, 256r+256) of each
     batch.
Host side shards/transposes/casts inputs (fp16) and reassembles the output.
"""

import sys

import numpy as np

if "/opt/trn_rl_repo" not in sys.path:
    sys.path.insert(0, "/opt/trn_rl_repo")

B, T, C, H, D = 2, 2048, 1024, 16, 64
NCORES = 8
HPC = H // NCORES          # heads per core = 2
CW = HPC * D               # per-core channel width = 128
KT = C // 128              # k tiles = 8
TT = T // 128              # t tiles = 16
SHARD = (B * T) // NCORES  # output rows per core = 512
SCALE = 1.0 / float(np.sqrt(C))

_CACHE = {}
LAST_EXEC_NS = None


def _build_nc():
    import concourse.mybir as mybir
    import concourse.tile as tile
    from concourse import bacc
    from concourse.masks import make_identity, make_upper_triangular

    f32 = mybir.dt.float32
    f16 = mybir.dt.float16

    nc = bacc.Bacc("TRN2", target_bir_lowering=False, debug=False,
                   num_devices=NCORES)

    xT = nc.dram_tensor("xT", [B, 128, KT * T], f16, kind="ExternalInput")
    wq = nc.dram_tensor("wq", [128, KT * CW], f16, kind="ExternalInput")
    wk = nc.dram_tensor("wk", [128, KT * CW], f16, kind="ExternalInput")
    wv = nc.dram_tensor("wv", [128, KT * CW], f16, kind="ExternalInput")
    wp = nc.dram_tensor("wp", [128, KT * C], f16, kind="ExternalInput")
    bq = nc.dram_tensor("bq", [CW, 1], f32, kind="ExternalInput")
    bk = nc.dram_tensor("bk", [CW, 1], f32, kind="ExternalInput")
    bv = nc.dram_tensor("bv", [CW, 1], f32, kind="ExternalInput")
    bp = nc.dram_tensor("bp", [1, C], f32, kind="ExternalInput")
    y = nc.dram_tensor("y", [SHARD, C], f32, kind="ExternalOutput")

    with tile.TileContext(nc) as tc:
        with (
            tc.tile_pool(name="const", bufs=1) as const,
            tc.tile_pool(name="dram", bufs=1, space="DRAM") as dram,
            tc.tile_pool(name="xtp", bufs=2) as xtp,
            tc.tile_pool(name="wqkv", bufs=1) as wqkvp,
            tc.tile_pool(name="qkv", bufs=1) as qkvp,
            tc.tile_pool(name="pt", bufs=5) as ptp,
            tc.tile_pool(name="otp", bufs=1) as otp,
            tc.tile_pool(name="sm", bufs=1) as smp,
            tc.tile_pool(name="proj", bufs=1) as projp,
            tc.tile_pool(name="ysb", bufs=2) as ysbp,
            tc.tile_pool(name="ps_gen", bufs=2, space="PSUM") as psg,
            tc.tile_pool(name="ps_s", bufs=2, space="PSUM") as ps_s,
            tc.tile_pool(name="ps_o", bufs=1, space="PSUM") as ps_o,
        ):
            # ---- collective warm-up (channel init overlaps compute) ----
            warm_i = dram.tile([8, 16], f32, name="warm_i")
            warm_o = dram.tile([8, 16], f32, name="warm_o")
            wtile = const.tile([8, 16], f32, name="wtile")
            nc.vector.memset(wtile[:], 0.0)
            nc.sync.dma_start(warm_i[:], wtile[:])
            nc.gpsimd.collective_compute(
                "AllToAll", mybir.AluOpType.bypass,
                replica_groups=[list(range(NCORES))],
                ins=[warm_i[:].opt()], outs=[warm_o[:].opt()],
            )

            # ---- weights first (small), then x chunks, then wp ----
            wq_sb = wqkvp.tile([128, KT * CW], f16, name="wq_sb")
            wk_sb = wqkvp.tile([128, KT * CW], f16, name="wk_sb")
            wv_sb = wqkvp.tile([128, KT * CW], f16, name="wv_sb")

            # x chunks: xt[b][j] holds tokens [512j, 512j+512) for all KT
            # c-chunks, a-major: [128, KT*512]
            xt_tiles = {0: [], 1: []}
            xT_r = [xT[b].rearrange("p (a t) -> p a t", a=KT) for b in range(B)]

            def load_x(b, j):
                xt = xtp.tile([128, KT * 512], f16, name=f"xt{b}_{j}",
                              tag=f"xt{j}")
                nc.sync.dma_start(
                    xt[:].rearrange("p (a w) -> p a w", a=KT),
                    xT_r[b][:, :, 512 * j : 512 * (j + 1)],
                )
                xt_tiles[b].append(xt)

            nc.sync.dma_start(wq_sb[:], wq[:])
            load_x(0, 0)
            load_x(0, 1)
            nc.sync.dma_start(wk_sb[:], wk[:])
            nc.sync.dma_start(wv_sb[:], wv[:])
            load_x(0, 2)
            load_x(0, 3)

            bq_t = const.tile([CW, 1], f32, name="bq_t")
            bk_t = const.tile([CW, 1], f32, name="bk_t")
            bv_t = const.tile([CW, 1], f32, name="bv_t")
            nc.sync.dma_start(bq_t[:], bq[:])
            nc.sync.dma_start(bk_t[:], bk[:])
            nc.sync.dma_start(bv_t[:], bv[:])
            bp_row = const.tile([1, C], f32, name="bp_row")
            nc.sync.dma_start(bp_row[:], bp[:])

            # batch-1 x and wp early so the HBM queues are quiet during the
            # batch-0 exchange
            for j in range(4):
                load_x(1, j)
            wp_sb = projp.tile([128, KT * C], f16, name="wp_sb")
            nc.sync.dma_start(wp_sb[:], wp[:])

            # ---- constants ----
            trimask = const.tile([128, 128], f16, name="trimask")
            make_upper_triangular(nc, trimask[:], val=1.0, diag=True)
            ident = const.tile([128, 128], f16, name="ident")
            make_identity(nc, ident[:])
            bpb = const.tile([128, C], f32, name="bpb")
            nc.gpsimd.partition_broadcast(bpb[:], bp_row[:])

            qT_sb, kT_sb, v_sb, ot_sb, r_all = {}, {}, {}, {}, {}
            # asymmetric exchange split: "A" carries all of batch 0 (slot r
            # cols 0:256 = b0 rows [256r, 256r+256)) plus batch 1's first
            # half (cols 256:384 = b1 rows [128r, 128r+128)); "B" carries
            # batch 1's second half (b1 rows [1024+128r, 1024+128r+128)).
            a2a_in = {
                "A": dram.tile([NCORES, 128, 384], f16, name="a2a_in_A"),
                "B": dram.tile([NCORES, 128, 128], f16, name="a2a_in_B"),
            }
            a2a_out = {
                "A": dram.tile([NCORES, 128, 384], f16, name="a2a_out_A"),
                "B": dram.tile([NCORES, 128, 128], f16, name="a2a_out_B"),
            }

            vT_sb = {}

            def emit_qkv_chunk(b, j):
                """qkv projection + v transposes for token chunk j of batch
                b. Emitted chunk-wise so attention chunk j can start as soon
                as its (causal) q/k/v prefix exists — attention work fills
                the x-DMA ramp instead of the PE idling."""
                if j == 0:
                    qT_sb[b] = qkvp.tile([128, T], f16, name=f"qT{b}")
                    kT_sb[b] = qkvp.tile([128, T], f16, name=f"kT{b}")
                    vT_sb[b] = qkvp.tile([128, T], f16, name=f"vT{b}")
                    v_sb[b] = []
                qT_b, kT_b, vT_b = qT_sb[b], kT_sb[b], vT_sb[b]
                xt = xt_tiles[b][j]
                for dst, w_sb, bias in (
                    (qT_b, wq_sb, bq_t), (kT_b, wk_sb, bk_t),
                    (vT_b, wv_sb, bv_t),
                ):
                    ps = psg.tile([128, 512], f32, name="ps_qk",
                                  tag="ps_gen")
                    for a in range(KT):
                        nc.tensor.matmul(
                            ps[:],
                            w_sb[:, CW * a : CW * (a + 1)],
                            xt[:, 512 * a : 512 * (a + 1)],
                            start=(a == 0), stop=(a == KT - 1),
                        )
                    # Scalar-engine eviction (Copy shares the Exp table, no
                    # reload): keeps the DVE queue clear at chunk boundaries
                    nc.scalar.activation(
                        dst[:, 512 * j : 512 * (j + 1)], ps[:],
                        mybir.ActivationFunctionType.Identity, bias=bias[:],
                    )
                for m in range(4 * j, 4 * j + 4):
                    vt = qkvp.tile([128, 2 * (D + 1)], f16,
                                   name=f"v{b}_{m}")
                    tps = psg.tile([128, 128], f16, name="ps_tr",
                                   tag="ps_gen")
                    nc.tensor.transpose(
                        tps[:], vT_b[:, 128 * m : 128 * (m + 1)], ident[:]
                    )
                    nc.vector.tensor_copy(
                        vt[:].rearrange("p (a m) -> p a m", a=2)[:, :, 0:D],
                        tps[:].rearrange("p (a m) -> p a m", a=2),
                    )
                    nc.vector.memset(vt[:, D : D + 1], 1.0)
                    nc.vector.memset(vt[:, 2 * D + 1 : 2 * D + 2], 1.0)
                    v_sb[b].append(vt)

            def emit_attn_chunk(b, j, last=False):
                """Attention for q-chunk j of batch b; stages its rows into
                the exchange buffers."""
                if j == 0:
                    ot = otp.tile([128, T], f16, name=f"ot{b}")
                    ot_sb[b] = ot
                    ra = smp.tile([1, 4096], f32, name=f"r_all{b}",
                                  tag=f"r_all{b}")
                    r_all[b] = ra
                ot, ra = ot_sb[b], r_all[b]
                o_ps = [
                    ps_o.tile([65, 512], f32, name=f"o{h}", tag=f"o{h}")
                    for h in range(2)
                ]
                ilast = 4 * (j + 1) - 1
                for i in range(4 * (j + 1)):
                    off = max(0, 128 * i - 512 * j)
                    s_ps = ps_s.tile([128, 1024], f32, name="s_ps", tag="s")
                    pt = ptp.tile([128, 1024], f16, name="pt", tag="pt")
                    for h in range(2):
                        nc.tensor.matmul(
                            s_ps[:, 512 * h + off : 512 * (h + 1)],
                            kT_sb[b][64 * h : 64 * h + 64,
                                     128 * i : 128 * (i + 1)],
                            qT_sb[b][64 * h : 64 * h + 64,
                                     512 * j + off : 512 * (j + 1)],
                            start=True, stop=True,
                        )
                    nc.scalar.activation(
                        pt[:].rearrange("p (g w) -> p g w", g=2)[:, :, off:512],
                        s_ps[:].rearrange("p (g w) -> p g w", g=2)[:, :, off:512],
                        mybir.ActivationFunctionType.Exp,
                        scale=SCALE,
                    )
                    if 4 * j <= i:
                        for h in range(2):
                            nc.vector.tensor_tensor(
                                pt[:, 512 * h + off : 512 * h + off + 128],
                                pt[:, 512 * h + off : 512 * h + off + 128],
                                trimask[:],
                                op=mybir.AluOpType.mult,
                            )
                    for h in range(2):
                        nc.tensor.matmul(
                            o_ps[h][0:65, off:512],
                            v_sb[b][i][:, (D + 1) * h : (D + 1) * (h + 1)],
                            pt[:, 512 * h + off : 512 * (h + 1)],
                            start=(i == 0), stop=(i == ilast),
                        )
                # rowsums + division + staging for chunk j
                for h in range(2):
                    idx = 2 * j + h
                    nc.vector.tensor_copy(
                        ra[0:1, 512 * idx : 512 * (idx + 1)],
                        o_ps[h][64:65, :],
                    )
                    if not last:
                        nc.vector.tensor_copy(
                            ot[64 * h : 64 * h + 64, 512 * j : 512 * (j + 1)],
                            o_ps[h][0:64, :],
                        )
                rs = ra[0:1, 1024 * j : 1024 * j + 1024]
                nc.vector.reciprocal_approx_fast(rs, rs)
                rb = smp.tile([128, 1024], f32, name="rb", tag="rb", bufs=2)
                nc.gpsimd.partition_broadcast(rb[:], rs)
                for h in range(2):
                    sl = ot[64 * h : 64 * h + 64, 512 * j : 512 * (j + 1)]
                    nc.vector.tensor_tensor(
                        sl,
                        # last chunk: evict+divide fused straight from PSUM
                        # (no successor waits on the o_ps banks)
                        o_ps[h][0:64, :] if last else sl,
                        rb[64 * h : 64 * h + 64, 512 * h : 512 * (h + 1)],
                        op=mybir.AluOpType.mult,
                    )
                src = ot[:, 512 * j : 512 * (j + 1)]
                if b == 0:
                    nc.sync.dma_start(
                        a2a_in["A"][2 * j : 2 * j + 2, :, 0:256]
                            .rearrange("r p w -> p r w"),
                        src.rearrange("p (r w) -> p r w", r=2),
                    )
                elif j < 2:
                    nc.sync.dma_start(
                        a2a_in["A"][4 * j : 4 * j + 4, :, 256:384]
                            .rearrange("r p w -> p r w"),
                        src.rearrange("p (r w) -> p r w", r=4),
                    )
                else:
                    nc.sync.dma_start(
                        a2a_in["B"][4 * (j - 2) : 4 * (j - 2) + 4, :, :]
                            .rearrange("r p w -> p r w"),
                        src.rearrange("p (r w) -> p r w", r=4),
                    )

            def emit_cc(key):
                nc.gpsimd.collective_compute(
                    "AllToAll", mybir.AluOpType.bypass,
                    replica_groups=[list(range(NCORES))],
                    ins=[a2a_in[key][:].opt()], outs=[a2a_out[key][:].opt()],
                )

            yts_sb = {}

            def emit_proj_load(key, m):
                """One fused DMA for all 8 lhsT blocks of a proj tile (a
                single Sync-engine trigger, so it can't convoy the queue)."""
                yts = projp.tile([128, KT * 128], f16, name=f"yts{key}_{m}",
                                 tag="yts", bufs=4)
                # scalar-engine DMA queue: keeps this cc-gated load from
                # head-of-line blocking the sync queue's staging DMAs
                nc.scalar.dma_start(
                    yts[:].rearrange("p (k w) -> p k w", k=KT),
                    a2a_out[key][:, :, 128 * m : 128 * (m + 1)]
                        .rearrange("k p w -> p k w"),
                )
                yts_sb[(key, m)] = yts

            def emit_proj(key, m, r0):
                """Output projection for my y rows [r0, r0+128) from column
                block m of exchange `key`."""
                yts = yts_sb[(key, m)]
                ysb = ysbp.tile([128, C], f32, name="ysb", tag="ysb")
                for n in range(2):
                    ps = psg.tile([128, 512], f32, name="ps_y", tag="ps_gen")
                    for k in range(KT):
                        nc.tensor.matmul(
                            ps[:],
                            yts[:, 128 * k : 128 * (k + 1)],
                            wp_sb[:, C * k + 512 * n : C * k + 512 * (n + 1)],
                            start=(k == 0), stop=(k == KT - 1),
                        )
                    nc.vector.tensor_tensor(
                        ysb[:, 512 * n : 512 * (n + 1)],
                        ps[:],
                        bpb[:, 512 * n : 512 * (n + 1)],
                        op=mybir.AluOpType.add,
                    )
                    # per-half y write: the first half ships while the
                    # second half's matmuls run
                    nc.scalar.dma_start(
                        y[r0 : r0 + 128, 512 * n : 512 * (n + 1)],
                        ysb[:, 512 * n : 512 * (n + 1)],
                    )

            # schedule: batch 0's exchange hides under batch 1's attention;
            # batch 0's projection overlaps batch 1's exchange on the PE, so
            # only batch 1's projection is fully exposed.
            for j in range(4):
                emit_qkv_chunk(0, j)
                emit_attn_chunk(0, j)
            emit_qkv_chunk(1, 0)
            emit_attn_chunk(1, 0)
            emit_qkv_chunk(1, 1)
            emit_attn_chunk(1, 1)
            emit_cc("A")
            emit_qkv_chunk(1, 2)
            emit_attn_chunk(1, 2)
            emit_qkv_chunk(1, 3)
            emit_attn_chunk(1, 3, last=True)
            emit_cc("B")
            # sim-time floor: the compile-time scheduler underestimates the
            # collectives, so without this it hoists proj work above the last
            # chunk's post-chain/staging on shared engine queues, delaying
            # the final exchange's doorbell by ~30us at runtime.
            with tc.tile_wait_until(1):
                emit_proj_load("A", 0)
                emit_proj_load("A", 1)
                emit_proj_load("A", 2)
                emit_proj_load("B", 0)
                emit_proj("A", 0, 0)
                emit_proj("A", 1, 128)
                emit_proj("A", 2, 256)
                emit_proj("B", 0, 384)

    nc.compile()
    return nc


def _get_nc():
    if "nc" not in _CACHE:
        _CACHE["nc"] = _build_nc()
    return _CACHE["nc"]


def kernel(x, W_attn, b_attn, W_proj, b_proj, _trace=False):
    global LAST_EXEC_NS
    from concourse.bass_utils import run_bass_kernel_spmd

    x = np.asarray(x, np.float32)
    W_attn = np.asarray(W_attn, np.float32)
    b_attn = np.asarray(b_attn, np.float32)
    W_proj = np.asarray(W_proj, np.float32)
    b_proj = np.asarray(b_proj, np.float32)

    def pmajor(w):  # [C, M] -> [128, KT*M], k-tile a at cols [a*M:(a+1)*M]
        m = w.shape[1]
        return np.ascontiguousarray(
            w.reshape(KT, 128, m).transpose(1, 0, 2).reshape(128, KT * m)
        ).astype(np.float16)

    xT = np.transpose(x, (0, 2, 1))  # [B, C, T]
    xT16 = np.ascontiguousarray(
        xT.reshape(B, KT, 128, T).transpose(0, 2, 1, 3).reshape(B, 128, KT * T)
    ).astype(np.float16)
    wp16 = pmajor(W_proj)
    bp_h = np.ascontiguousarray(b_proj).reshape(1, C)

    in_maps = []
    for c in range(NCORES):
        s = slice(CW * c, CW * (c + 1))
        in_maps.append({
            "xT": xT16,
            "wq": pmajor(W_attn[:, s]),
            "wk": pmajor(W_attn[:, C:][:, s]),
            "wv": pmajor(W_attn[:, 2 * C:][:, s]),
            "wp": wp16,
            "bq": np.ascontiguousarray(b_attn[s]).reshape(CW, 1),
            "bk": np.ascontiguousarray(b_attn[C:][s]).reshape(CW, 1),
            "bv": np.ascontiguousarray(b_attn[2 * C:][s]).reshape(1, CW),
            "bp": bp_h,
        })

    nc = _get_nc()
    res = run_bass_kernel_spmd(nc, in_maps, list(range(NCORES)), trace=_trace)
    LAST_EXEC_NS = res.exec_time_ns

    # core c's y rows: [0,256) = batch 0 tokens [256c, 256c+256);
    # [256,384) = batch 1 tokens [128c, 128c+128);
    # [384,512) = batch 1 tokens [1024+128c, 1024+128c+128).
    out = np.empty((B, T, C), np.float32)
    for c in range(NCORES):
        yc = res.results[c]["y"]
        out[0, 256 * c : 256 * (c + 1), :] = yc[0:256, :]
        out[1, 128 * c : 128 * (c + 1), :] = yc[256:384, :]
        out[1, 1024 + 128 * c : 1024 + 128 * (c + 1), :] = yc[384:512, :]
    return out
